# revision 16
# baseline (speedup 1.0000x reference)
"""
Multi-head attention (B=2, S=2048, D=1024, H=16, causal mask) on 8 Trainium2
NeuronCores via Bass/Tile.

Sharding: batch x heads (data + tensor parallel) -- core c owns batch c//4
and the 4 heads [4*(c%4), 4*(c%4)+4) of that batch.  Each core reads only
its batch's activations, computes Q/K/V projections for its 256 features,
runs causal attention for its 4 heads (as two 128-feature head-pair groups),
and produces a partial output projection [2048, 1024].  The host sums the
partials per batch (plus the last-block g1 partial `out2`) and adds the
output bias.

Schedule (v3): software-pipelined around the pacing engines:
  * PE inner loop uses a one-step score LOOKAHEAD: S(ki+1) is emitted
    before PV(ki), so the in-order PE queue never head-of-line blocks on
    the ACT exp of the current tile.
  * Projections run at 512-token QUARTER granularity, woven into the
    attention blocks as PE filler.  The q-part of quarter Q+1 must finish
    inside block Q; the k/v parts may SPILL into block Q+1 (flushed by a
    guard before the stream first reads that quarter).  Filler yields
    "safe" markers at points with no open PSUM accumulation; other users
    of the same PSUM pool only emit at safe points (deadlock avoidance
    for the in-order engine queues).
  * x tiles arrive via few, large rearranged DMAs (DMA-issue instructions
    cost ~0.6us each on the issuing engine, so issue count is the real
    input-stream pacer).  Quarter 0 streams at finer granularity so the
    first projections can start as data trickles in.
  * The deferred output projection of block qj is emitted at ki==1 of
    block qj+1's first stream; the last block's outproj is split per
    group (g0's partial store lands during g1's stream; g1's partial
    goes to `out2`), halving the end-of-kernel drain.
  * ACT is exp-only during attention (projection psum drains ride DVE,
    causal masking rides GpSimd, which cannot touch PSUM but can mask
    SBUF pt tiles).

On-chip layouts (per core):
  Q_T, K_T : [128 feats (2 heads x 64), group g, 512 tokens] per quarter
  V        : vaug [128 tokens, g*4+ktile, head, 65]; col 64 == 1.0
             so the P@V matmul also produces the softmax row sums
  S_T      : scores^T tiles [128 keys, q]
  softmax  : exp on ACT (scale=1/8 folded; no max-subtraction needed),
             sums via the ones column of V, fast approx-reciprocal on
             DVE, partition-broadcast via a K=1 ones matmul on PE,
             normalize fused into the psum->sbuf move.
  out-proj : both heads of a group packed into one [128, 512] otn tile;
             the two groups accumulate into the same psum tile (except
             the split last block).
"""

import os
import sys

for _p in ("/opt/trn_rl_repo", "/root/.axon_site/_ro/trn_rl_repo"):
    if os.path.isdir(_p) and _p not in sys.path:
        sys.path.insert(0, _p)

import numpy as np
import ml_dtypes
from contextlib import ExitStack

import concourse.bass as bass
import concourse.tile as tile
from concourse import bacc
from concourse import mybir
from concourse.dve_ops import (
    RECIP_APPROX_FAST_CONSTS,
    RECIPROCAL_APPROX_FAST,
)

B, S, D, H = 2, 2048, 1024, 16
DK = D // H            # 64
NCORES = 8
BGROUPS = 2            # batch groups
CPB = NCORES // BGROUPS    # cores per batch = 4
HPC_TOT = H // CPB     # 4 heads per core
G = 2                  # head-pair groups per core
HPC = HPC_TOT // G     # 2 heads per group
DH = HPC_TOT * DK      # 256 features per core
SCALE = 1.0 / np.sqrt(DK)  # 0.125

F32 = mybir.dt.float32
F32R = mybir.dt.float32r
BF16 = mybir.dt.bfloat16


class _Filler:
    """Priority queue of projection generators used as PE filler.

    `now` entries must finish within the current attention block;
    `spill` entries may run ahead opportunistically and roll over.
    Each entry is (quarter_tag, generator).  Generators yield "safe"
    when they hold no open PSUM accumulation."""

    def __init__(self):
        self.now = []
        self.spill = []
        self.marker = "safe"

    def _pump_one(self):
        while True:
            if self.now:
                src, gen = self.now, self.now[0][1]
            elif self.spill:
                src, gen = self.spill, self.spill[0][1]
            else:
                return False
            try:
                self.marker = next(gen)
                return True
            except StopIteration:
                self.marker = "safe"
                src.pop(0)

    def pump(self, n):
        for _ in range(n):
            if not self._pump_one():
                return

    def pump_to_safe(self):
        while self.marker != "safe":
            if not self._pump_one():
                return

    def flush_now(self):
        while self.now:
            if not self._pump_one():
                break
        self.pump_to_safe()

    def guard(self, qj):
        """Finish every now-generator tagged <= qj (its outputs are about
        to be read by the stream)."""
        while self.now and self.now[0][0] <= qj:
            gen = self.now[0][1]
            try:
                while True:
                    self.marker = next(gen)
            except StopIteration:
                self.marker = "safe"
                self.now.pop(0)

    def rotate(self):
        self.flush_now()
        self.now = self.spill
        self.spill = []


def build_kernel(seq=S, mode="causal", xdt=BF16, dbg=False):
    """Build the per-core Bass program.  Identical program on all cores;
    per-core batch/head slices arrive as data."""
    T = seq
    mmdt = F32R if xdt == F32 else xdt   # attention matmul dtype
    pjdt = BF16                          # projection matmul dtype
    KC = D // 128               # 8 contraction chunks for projections
    NQ = seq // 512             # 4 query blocks == 4 token quarters
    NKT = seq // 128            # 16 k tiles of 128
    HKQ = 4                     # k tiles per quarter
    nc = bacc.Bacc()

    xq = nc.declare_dram_parameter("xq", [D, T], pjdt, isOutput=False)
    xk = nc.declare_dram_parameter("xk", [D, T], pjdt, isOutput=False)
    xv = nc.declare_dram_parameter("xv", [D, T], pjdt, isOutput=False)
    wq = nc.declare_dram_parameter("wq", [D, DH], pjdt, isOutput=False)
    wk = nc.declare_dram_parameter("wk", [D, DH], pjdt, isOutput=False)
    wv = nc.declare_dram_parameter("wv", [D, DH], pjdt, isOutput=False)
    wqb = nc.declare_dram_parameter("wqb", [128, G], F32, isOutput=False)
    wkb = nc.declare_dram_parameter("wkb", [128, G], F32, isOutput=False)
    wvb = nc.declare_dram_parameter("wvb", [128, G], F32, isOutput=False)
    wo = nc.declare_dram_parameter("wo", [DH, D], pjdt, isOutput=False)
    tri = nc.declare_dram_parameter("tri", [128, 128], mmdt, isOutput=False)
    idn = nc.declare_dram_parameter("idn", [128, 128], pjdt, isOutput=False)
    onesm = nc.declare_dram_parameter("onesm", [128, 512], mmdt, isOutput=False)
    onesr = nc.declare_dram_parameter("onesr", [1, DK], F32R, isOutput=False)
    madd = None
    if mode == "general":
        madd = nc.declare_dram_parameter("madd", [seq, seq], F32, isOutput=False)
    out = nc.declare_dram_parameter("out", [T, D], BF16, isOutput=True)
    # last block's group-1 partial (host adds it into rows [T-512, T))
    out2 = nc.declare_dram_parameter("out2", [512, D], BF16, isOutput=True)

    with tile.TileContext(nc) as tc, ExitStack() as ctx:
        persist = ctx.enter_context(tc.tile_pool(name="persist", bufs=1))
        wpool = ctx.enter_context(tc.tile_pool(name="wpool", bufs=1))
        xs0 = ctx.enter_context(tc.tile_pool(name="xs0", bufs=8))
        xsH = ctx.enter_context(tc.tile_pool(name="xsH", bufs=4))
        xsB = ctx.enter_context(tc.tile_pool(name="xsB", bufs=6))
        vts = ctx.enter_context(tc.tile_pool(name="vts", bufs=3))
        ptp = ctx.enter_context(tc.tile_pool(name="ptp", bufs=6))
        otn_p = ctx.enter_context(tc.tile_pool(name="otn", bufs=6))
        rc_p = ctx.enter_context(tc.tile_pool(name="rc", bufs=6))
        out_p = ctx.enter_context(tc.tile_pool(name="outp", bufs=6))
        mk_p = None
        if mode == "general":
            mk_p = ctx.enter_context(tc.tile_pool(name="mk", bufs=4))
        # PSUM: st2 2 bufs x 2 banks + otps 2 x 1 + po 2 x 1 = 8 banks
        st2 = ctx.enter_context(
            tc.tile_pool(name="st2", bufs=2, space=bass.MemorySpace.PSUM))
        otps = ctx.enter_context(
            tc.tile_pool(name="otps", bufs=2, space=bass.MemorySpace.PSUM))
        po = ctx.enter_context(
            tc.tile_pool(name="po", bufs=2, space=bass.MemorySpace.PSUM))

        # ---------------- persistent tiles ----------------
        # per-(quarter, group) tiles: attention consumers wait only on the
        # group slice they actually read
        qt_c = [[persist.tile([128, 512], mmdt, name=f"qt{i}g{g}")
                 for g in range(G)] for i in range(NQ)]
        kt_c = [[persist.tile([128, 512], mmdt, name=f"kt{i}g{g}")
                 for g in range(G)] for i in range(NQ)]
        # V augmented: [128 tokens, g*HKQ + ktile, head-in-group, 65]
        vaug_c = [persist.tile([128, G * HKQ, HPC, DK + 1], mmdt,
                               name=f"vaug{i}") for i in range(NQ)]
        wo_sb = persist.tile([128, G, D], pjdt)
        tri_sb = persist.tile([128, 128], mmdt)
        ident = persist.tile([128, 128], pjdt)
        ones_sb = persist.tile([128, 512], mmdt)
        onesr_sb = persist.tile([1, DK], F32R)

        # ---------------- weight / constant DMAs ----------------
        # wv rides the sync queue AHEAD of the x stream (V projects first
        # in the pre-attention quarter); everything else rides the gpsimd
        # queue, ordered by first use.
        wsrc_d = {"q": (xq, wq, qt_c), "k": (xk, wk, kt_c),
                  "v": (xv, wv, None)}
        w_sb = {}
        wb_sb = {}
        for name in ("q", "k", "v"):
            w_sb[name] = wpool.tile([128, KC, DH], pjdt, tag=f"w{name}",
                                    name=f"w{name}")
            wb_sb[name] = wpool.tile([128, G], F32, tag=f"wb{name}",
                                     name=f"wb{name}")
        nc.sync.dma_start(
            out=w_sb["v"],
            in_=wv[:, :].rearrange("(c p) n -> p c n", p=128))
        for name, bsrc in (("v", wvb), ("k", wkb), ("q", wqb)):
            nc.gpsimd.dma_start(out=wb_sb[name], in_=bsrc[:, :])
        nc.gpsimd.dma_start(out=ones_sb, in_=onesm[:, :])
        nc.gpsimd.dma_start(out=ident, in_=idn[:, :])
        nc.gpsimd.dma_start(
            out=w_sb["k"],
            in_=wk[:, :].rearrange("(c p) n -> p c n", p=128))
        nc.gpsimd.dma_start(out=tri_sb, in_=tri[:, :])
        nc.gpsimd.dma_start(
            out=w_sb["q"],
            in_=wq[:, :].rearrange("(c p) n -> p c n", p=128))
        nc.gpsimd.dma_start(out=onesr_sb, in_=onesr[:, :])
        nc.gpsimd.dma_start(
            out=wo_sb, in_=wo[:, :].rearrange("(g p) n -> p g n", p=128))

        # ---------------- x input streaming ----------------
        xt_access = {}       # (name, Q) -> fn(c) -> AP of chunk c

        def emit_x_dmas(Q, parts):
            """Issue quarter Q's x DMAs on the sync queue.  Quarter 0 is
            split finer (per-chunk / half) so the first projections can
            start while data streams in; later quarters use one large
            rearranged DMA per tensor to save issue time."""
            for name in parts:
                xsrc = wsrc_d[name][0]
                if Q == 0 and name == "v":
                    ts = []
                    for c in range(KC):
                        t = xs0.tile([128, 512], pjdt, tag="x0")
                        nc.sync.dma_start(
                            out=t, in_=xsrc[c * 128:(c + 1) * 128, 0:512])
                        ts.append(t)
                    xt_access[(name, Q)] = lambda c, ts=ts: ts[c]
                elif Q == 0:
                    # xq rides the gpsimd queue (after the weights) so it
                    # streams in parallel with sync's xv/xk
                    deng = nc.gpsimd if name == "q" else nc.sync
                    hs = []
                    for hh in range(2):
                        t = xsH.tile([128, KC // 2, 512], pjdt, tag="xh")
                        deng.dma_start(
                            out=t,
                            in_=xsrc[hh * 512:(hh + 1) * 512, 0:512]
                            .rearrange("(c p) t -> p c t", p=128))
                        hs.append(t)
                    xt_access[(name, Q)] = (
                        lambda c, hs=hs: hs[c // 4][:, c % 4, :])
                else:
                    t = xsB.tile([128, KC, 512], pjdt, tag="xb")
                    nc.sync.dma_start(
                        out=t,
                        in_=xsrc[:, Q * 512:(Q + 1) * 512]
                        .rearrange("(c p) t -> p c t", p=128))
                    xt_access[(name, Q)] = lambda c, t=t: t[:, c, :]

        def proj_gen(Q, parts, on_act):
            """Generator emitting quarter Q's projections one unit at a
            time.  Yields "safe" where no PSUM accumulation is open."""
            for name in parts:
                wt, bt = w_sb[name], wb_sb[name]
                xap = xt_access[(name, Q)]
                vtile = None
                if name == "v":
                    vtile = vts.tile([128, G, 512], pjdt, tag="vt")
                for g in range(G):
                    ps = po.tile([128, 512], F32, tag="po")
                    for c in range(KC):
                        nc.tensor.matmul(
                            ps, wt[:, c, g * 128:(g + 1) * 128], xap(c),
                            start=(c == 0), stop=(c == KC - 1))
                        yield None
                    if name == "v":
                        tgt = vtile[:, g, :]
                    else:
                        tgt = wsrc_d[name][2][Q][g][:, :]
                    if on_act:
                        nc.scalar.activation(
                            tgt, ps, mybir.ActivationFunctionType.Identity,
                            bias=bt[:, g:g + 1])
                    else:
                        # GpSimd cannot read PSUM; DVE drains the filler
                        nc.vector.tensor_scalar_add(tgt, ps, bt[:, g:g + 1])
                    yield "safe"
                if name == "v":
                    nc.vector.tensor_copy(
                        vaug_c[Q][:, :, :, DK:DK + 1],
                        ones_sb[:, 0:G * HKQ * HPC])
                    yield "safe"
                    for g in range(G):
                        for i in range(HKQ):
                            trp = po.tile([128, HPC, DK], pjdt, tag="po")
                            nc.tensor.transpose(
                                trp, vtile[:, g, i * 128:(i + 1) * 128],
                                ident)
                            yield None
                            nc.vector.tensor_copy(
                                vaug_c[Q][:, g * HKQ + i, :, 0:DK], trp)
                            yield "safe"

        # ---------------- attention ----------------
        def emit_scores(qj, g, ki):
            """Score matmuls for one 128-key tile; returns (st, off)."""
            off = 4 * (ki - 4 * qj) * 32 if (mode == "causal" and ki >= 4 * qj) else 0
            kh, kbase = ki // HKQ, (ki % HKQ) * 128
            st = st2.tile([128, 1024], F32, tag="st2")
            for h in range(HPC):
                nc.tensor.matmul(
                    st[:, h * 512 + off:(h + 1) * 512],
                    kt_c[kh][g][h * DK:(h + 1) * DK, kbase:kbase + 128],
                    qt_c[qj][g][h * DK:(h + 1) * DK, off:512],
                    start=True, stop=True,
                    tile_position=(h * DK, 0))
            if mode == "general":
                mt = mk_p.tile([128, 512], F32, tag="mk")
                nc.sync.dma_start(
                    out=mt,
                    in_=madd[ki * 128:(ki + 1) * 128,
                             qj * 512:(qj + 1) * 512])
                for h in range(HPC):
                    nc.vector.tensor_add(
                        st[:, h * 512:(h + 1) * 512],
                        st[:, h * 512:(h + 1) * 512], mt)
            return st, off

        def attn_ki_stream(qj, g, ots, filler, pending):
            """Pipelined score/exp/PV stream for one (qj, group).
            Emits S(ki+1) before PV(ki) so the PE never waits on exp;
            pumps filler between steps; emits the deferred `pending` job
            at ki==1 (at a filler-safe point)."""
            n_k = 4 * qj + 4 if mode == "causal" else NKT
            ots[g] = [otps.tile([DK + 1, 512], F32, tag="ot",
                                name=f"ot{_h}") for _h in range(HPC)]
            ot = ots[g]
            pend_s = emit_scores(qj, g, 0)
            for ki in range(n_k):
                st, off = pend_s
                pt = ptp.tile([128, 1024], mmdt, tag="pt")
                if off == 0:
                    nc.scalar.activation(
                        pt, st, mybir.ActivationFunctionType.Exp, scale=SCALE)
                else:
                    for h in range(HPC):
                        lo = h * 512
                        nc.scalar.activation(
                            pt[:, lo + off:lo + 512], st[:, lo + off:lo + 512],
                            mybir.ActivationFunctionType.Exp, scale=SCALE)
                # lookahead: next scores enter the PE queue before PV(ki)
                if ki + 1 < n_k:
                    if ki + 1 == 4 * qj:
                        # about to read this block's own quarter
                        filler.guard(qj)
                    pend_s = emit_scores(qj, g, ki + 1)
                if mode == "causal" and ki >= 4 * qj:
                    # pt/tri are SBUF-only -> GpSimd masks them, keeping
                    # DVE free for the psum drains
                    for h in range(HPC):
                        lo = h * 512 + off
                        nc.gpsimd.tensor_mul(
                            pt[:, lo:lo + 128], pt[:, lo:lo + 128], tri_sb)
                kh, vs = ki // HKQ, ki % HKQ
                for h in range(HPC):
                    nc.tensor.matmul(
                        ot[h][:, off:512] if off else ot[h],
                        vaug_c[kh][:, g * HKQ + vs, h, :],
                        pt[:, h * 512 + off:(h + 1) * 512],
                        start=(ki == 0), stop=(ki == n_k - 1))
                filler.pump(3)
                if ki == 1 and pending[0] is not None:
                    filler.pump_to_safe()
                    pending[0]()
                    pending[0] = None

        def attn_norm(qj, g, ots, filler):
            """Normalize one finished group; frees its PV psum tiles.
            Filler is pumped between the reciprocal chain and the rbc
            matmuls so the PE has work during the DVE latency."""
            ot = ots[g]
            rrows = []
            for h in range(HPC):
                srow = rc_p.tile([1, 512], F32R, tag="srow",
                                 name=f"srow{h}")
                nc.vector.tensor_copy(srow, ot[h][DK:DK + 1, :])
                rrow_r = rc_p.tile([1, 512], F32R, tag="rrow_r",
                                   name=f"rrow_r{h}")
                _rc = RECIP_APPROX_FAST_CONSTS
                nc.vector._custom_dve(
                    RECIPROCAL_APPROX_FAST, out=rrow_r, in0=srow,
                    s0=_rc["s0"], s1=_rc["s1"], imm2=_rc["imm2"])
                rrows.append(rrow_r)
            filler.pump(4)
            filler.pump_to_safe()
            rbcs = []
            for h in range(HPC):
                rbc_ps = po.tile([DK, 512], F32, tag="po")
                nc.tensor.matmul(rbc_ps, onesr_sb, rrows[h],
                                 start=True, stop=True)
                rbc = rc_p.tile([DK, 512], F32, tag="rbc",
                                name=f"rbc{h}")
                # ACT is idle at group boundaries; draining rbc there keeps
                # the serial DVE chain short
                nc.scalar.copy(rbc, rbc_ps)
                rbcs.append(rbc)
            otn = otn_p.tile([128, 512], pjdt, tag="otn", name=f"otn{g}")
            for h in range(HPC):
                nc.vector.tensor_mul(
                    otn[h * DK:(h + 1) * DK, :], ot[h][0:DK, :], rbcs[h])
            return otn

        def attn_outproj(qj, otn_g):
            """Combined 256-feature output projection (blocks 0..NQ-2)."""
            qbase = qj * 512
            for qb in range(4):
                pts = [po.tile([128, 512], F32, tag="po", name=f"pts{_n}")
                       for _n in range(2)]
                for n in range(2):
                    for g in range(G):
                        nc.tensor.matmul(
                            pts[n],
                            otn_g[g][:, qb * 128:(qb + 1) * 128],
                            wo_sb[:, g, n * 512:(n + 1) * 512],
                            start=(g == 0), stop=(g == G - 1))
                ob = out_p.tile([128, 1024], BF16, tag="ob")
                # both copies on DVE: this runs inside an exp-paced stream,
                # so ACT must stay exp-only
                nc.vector.tensor_copy(ob[:, 0:512], pts[0])
                nc.vector.tensor_copy(ob[:, 512:1024], pts[1])
                nc.gpsimd.dma_start(
                    out=out[qbase + qb * 128:qbase + (qb + 1) * 128, :],
                    in_=ob)

        def outproj_partial(qj, otn, g, part):
            """Single-group output projection for the LAST block.  part 0
            stores g0's partial into `out` rows (g1's lands in `out2`;
            the host adds them), so g0's drain overlaps g1's stream."""
            qbase = qj * 512
            for qb in range(4):
                pts = [po.tile([128, 512], F32, tag="po", name=f"pp{_n}")
                       for _n in range(2)]
                for n in range(2):
                    nc.tensor.matmul(
                        pts[n],
                        otn[:, qb * 128:(qb + 1) * 128],
                        wo_sb[:, g, n * 512:(n + 1) * 512],
                        start=True, stop=True)
                ob = out_p.tile([128, 1024], BF16, tag="ob")
                if part == 0:
                    # during g1's stream: DVE-only copies, ACT stays exp-only
                    nc.vector.tensor_copy(ob[:, 0:512], pts[0])
                    nc.vector.tensor_copy(ob[:, 512:1024], pts[1])
                    nc.gpsimd.dma_start(
                        out=out[qbase + qb * 128:qbase + (qb + 1) * 128, :],
                        in_=ob)
                else:
                    # end of kernel: ACT is idle, split copies across engines
                    nc.vector.tensor_copy(ob[:, 0:512], pts[0])
                    nc.scalar.copy(ob[:, 512:1024], pts[1])
                    eng = nc.sync if qb % 2 == 0 else nc.gpsimd
                    eng.dma_start(
                        out=out2[qb * 128:(qb + 1) * 128, :], in_=ob)

        def attn_block(qj, filler, pending):
            last = qj == NQ - 1
            ots = {}
            otn_g = []
            attn_ki_stream(qj, 0, ots, filler, pending)
            filler.pump_to_safe()
            otn_g.append(attn_norm(qj, 0, ots, filler))
            if last:
                otn0 = otn_g[0]
                pending[0] = lambda: outproj_partial(qj, otn0, 0, 0)
            attn_ki_stream(qj, 1, ots, filler, pending)
            filler.pump_to_safe()
            otn_g.append(attn_norm(qj, 1, ots, filler))
            if last:
                if pending[0] is not None:     # n_k tiny safeguard
                    pending[0]()
                    pending[0] = None
                outproj_partial(qj, otn_g[1], 1, 1)
            else:
                pending[0] = lambda: attn_outproj(qj, otn_g)

        # ---------------- schedule ----------------
        emit_x_dmas(0, ("v", "k", "q"))
        boot = _Filler()
        boot.now = [(0, proj_gen(0, ("v", "k", "q"), on_act=True))]
        boot.flush_now()
        filler = _Filler()
        pending = [None]
        for qj in range(NQ):
            if qj + 1 < NQ:
                emit_x_dmas(qj + 1, ("q", "k", "v"))
                filler.now.append(
                    (qj + 1, proj_gen(qj + 1, ("q",), on_act=False)))
                filler.spill.append(
                    (qj + 1, proj_gen(qj + 1, ("k", "v"), on_act=False)))
            attn_block(qj, filler, pending)
            filler.rotate()
        if pending[0] is not None:
            pending[0]()
    nc.compile()
    return nc


def _np_dt(xdt):
    return np.float32 if xdt == F32 else ml_dtypes.bfloat16


def make_core_inputs(query, key, value, mask, Wq, bq, Wk, bk, Wv, bv, Wo, bo,
                     seq=S, mode="causal", xdt=BF16):
    """Host-side sharding: returns list of per-core input dicts."""
    ndt = _np_dt(xdt)
    pdt = ml_dtypes.bfloat16
    xq_b = [np.ascontiguousarray(query[b].reshape(seq, D).T.astype(pdt))
            for b in range(B)]
    xk_b = [np.ascontiguousarray(key[b].reshape(seq, D).T.astype(pdt))
            for b in range(B)]
    xv_b = [np.ascontiguousarray(value[b].reshape(seq, D).T.astype(pdt))
            for b in range(B)]
    tri = np.ascontiguousarray(np.triu(np.ones((128, 128), np.float32))).astype(ndt)
    in_maps = []
    for c in range(NCORES):
        b = c // CPB
        hq = c % CPB
        hsl = slice(DH * hq, DH * (hq + 1))
        m = {
            "xq": xq_b[b], "xk": xk_b[b], "xv": xv_b[b],
            "wq": np.ascontiguousarray(Wq[hsl, :].T.astype(pdt)),
            "wk": np.ascontiguousarray(Wk[hsl, :].T.astype(pdt)),
            "wv": np.ascontiguousarray(Wv[hsl, :].T.astype(pdt)),
            "wqb": np.ascontiguousarray(
                bq[hsl].astype(np.float32).reshape(G, 128).T),
            "wkb": np.ascontiguousarray(
                bk[hsl].astype(np.float32).reshape(G, 128).T),
            "wvb": np.ascontiguousarray(
                bv[hsl].astype(np.float32).reshape(G, 128).T),
            "wo": np.ascontiguousarray(Wo[:, hsl].T.astype(pdt)),
            "tri": tri,
            "idn": np.ascontiguousarray(np.eye(128, dtype=np.float32)).astype(pdt),
            "onesm": np.ones((128, 512), ndt),
            "onesr": np.ones((1, DK), np.float32),
        }
        if mode == "general":
            m["madd"] = np.ascontiguousarray(
                np.where(np.asarray(mask)[0, 0].T == 0, np.float32(-1e30),
                         np.float32(0.0)).astype(np.float32))
        in_maps.append(m)
    return in_maps


def detect_mode(mask, seq=S):
    m = np.asarray(mask)[0, 0]
    if (m == np.tril(np.ones((seq, seq), m.dtype))).all():
        return "causal"
    if (m == 1).all():
        return "ones"
    return "general"


_NC_CACHE = {}


def kernel(query, key, value, mask, Wq, bq, Wk, bk, Wv, bv, Wo, bo,
           xdt=BF16, trace=False):
    from concourse.bass_utils import run_bass_kernel_spmd

    query = np.asarray(query)
    mode = detect_mode(mask)
    key_ = (S, mode, xdt)
    if key_ not in _NC_CACHE:
        _NC_CACHE[key_] = build_kernel(seq=S, mode=mode, xdt=xdt)
    nc = _NC_CACHE[key_]
    in_maps = make_core_inputs(
        np.asarray(query), np.asarray(key), np.asarray(value), mask,
        np.asarray(Wq), np.asarray(bq), np.asarray(Wk), np.asarray(bk),
        np.asarray(Wv), np.asarray(bv), np.asarray(Wo), np.asarray(bo),
        seq=S, mode=mode, xdt=xdt)
    res = run_bass_kernel_spmd(nc, in_maps, core_ids=list(range(NCORES)),
                               trace=trace)
    acc = np.zeros((B, S, D), np.float64)
    for c, r in enumerate(res.results):
        acc[c // CPB] += r["out"].astype(np.float64)
        acc[c // CPB, S - 512:S, :] += r["out2"].astype(np.float64)
    acc += np.asarray(bo).astype(np.float64)[None, None, :]
    out = acc.astype(np.float32)
    if trace:
        kernel.last_results = res
    return out


# revision 18
# speedup vs baseline: 1.0426x; 1.0426x over previous
"""
Multi-head attention (B=2, S=2048, D=1024, H=16, causal mask) on 8 Trainium2
NeuronCores via Bass/Tile.

Sharding: batch x heads (data + tensor parallel) -- core c owns batch c//4
and the 4 heads [4*(c%4), 4*(c%4)+4) of that batch.  Each core reads only
its batch's activations, computes Q/K/V projections for its 256 features,
runs causal attention for its 4 heads (as two 128-feature head-pair groups),
and produces a partial output projection [2048, 1024].  The host sums the
partials per batch (plus the last-block g1 partial `out2`) and adds the
output bias.

Schedule (v3): software-pipelined around the pacing engines:
  * PE inner loop uses a one-step score LOOKAHEAD: S(ki+1) is emitted
    before PV(ki), so the in-order PE queue never head-of-line blocks on
    the ACT exp of the current tile.
  * Projections run at 512-token QUARTER granularity, woven into the
    attention blocks as PE filler.  The q-part of quarter Q+1 must finish
    inside block Q; the k/v parts may SPILL into block Q+1 (flushed by a
    guard before the stream first reads that quarter).  Filler yields
    "safe" markers at points with no open PSUM accumulation; other users
    of the same PSUM pool only emit at safe points (deadlock avoidance
    for the in-order engine queues).
  * x tiles arrive via few, large rearranged DMAs (DMA-issue instructions
    cost ~0.6us each on the issuing engine, so issue count is the real
    input-stream pacer).  Quarter 0 streams at finer granularity so the
    first projections can start as data trickles in.
  * The deferred output projection of block qj is emitted at ki==1 of
    block qj+1's first stream; the last block's outproj is split per
    group (g0's partial store lands during g1's stream; g1's partial
    goes to `out2`), halving the end-of-kernel drain.
  * ACT is exp-only during attention (projection psum drains ride DVE,
    causal masking rides GpSimd, which cannot touch PSUM but can mask
    SBUF pt tiles).

On-chip layouts (per core):
  Q_T, K_T : [128 feats (2 heads x 64), group g, 512 tokens] per quarter
  V        : vaug [128 tokens, g*4+ktile, head, 65]; col 64 == 1.0
             so the P@V matmul also produces the softmax row sums
  S_T      : scores^T tiles [128 keys, q]
  softmax  : exp on ACT (scale=1/8 folded; no max-subtraction needed),
             sums via the ones column of V, fast approx-reciprocal on
             DVE, partition-broadcast via a K=1 ones matmul on PE,
             normalize fused into the psum->sbuf move.
  out-proj : both heads of a group packed into one [128, 512] otn tile;
             the two groups accumulate into the same psum tile (except
             the split last block).
"""

import os
import sys

for _p in ("/opt/trn_rl_repo", "/root/.axon_site/_ro/trn_rl_repo"):
    if os.path.isdir(_p) and _p not in sys.path:
        sys.path.insert(0, _p)

import numpy as np
import ml_dtypes
from contextlib import ExitStack

import concourse.bass as bass
import concourse.tile as tile
from concourse import bacc
from concourse import mybir
from concourse.dve_ops import (
    RECIP_APPROX_FAST_CONSTS,
    RECIPROCAL_APPROX_FAST,
)

B, S, D, H = 2, 2048, 1024, 16
DK = D // H            # 64
NCORES = 8
BGROUPS = 2            # batch groups
CPB = NCORES // BGROUPS    # cores per batch = 4
HPC_TOT = H // CPB     # 4 heads per core
G = 2                  # head-pair groups per core
HPC = HPC_TOT // G     # 2 heads per group
DH = HPC_TOT * DK      # 256 features per core
SCALE = 1.0 / np.sqrt(DK)  # 0.125

F32 = mybir.dt.float32
F32R = mybir.dt.float32r
BF16 = mybir.dt.bfloat16


class _Filler:
    """Priority queue of projection generators used as PE filler.

    `now` entries must finish within the current attention block;
    `spill` entries may run ahead opportunistically and roll over.
    Each entry is (quarter_tag, generator).  Generators yield "safe"
    when they hold no open PSUM accumulation."""

    def __init__(self):
        self.now = []
        self.spill = []
        self.marker = "safe"

    def _pump_one(self):
        while True:
            if self.now:
                src, gen = self.now, self.now[0][1]
            elif self.spill:
                src, gen = self.spill, self.spill[0][1]
            else:
                return False
            try:
                self.marker = next(gen)
                return True
            except StopIteration:
                self.marker = "safe"
                src.pop(0)

    def pump(self, n):
        for _ in range(n):
            if not self._pump_one():
                return

    def pump_to_safe(self):
        while self.marker != "safe":
            if not self._pump_one():
                return

    def flush_now(self):
        while self.now:
            if not self._pump_one():
                break
        self.pump_to_safe()

    def guard(self, qj):
        """Finish every now-generator tagged <= qj (its outputs are about
        to be read by the stream)."""
        while self.now and self.now[0][0] <= qj:
            gen = self.now[0][1]
            try:
                while True:
                    self.marker = next(gen)
            except StopIteration:
                self.marker = "safe"
                self.now.pop(0)

    def rotate(self):
        self.flush_now()
        self.now = self.spill
        self.spill = []


def build_kernel(seq=S, mode="causal", xdt=BF16, dbg=False):
    """Build the per-core Bass program.  Identical program on all cores;
    per-core batch/head slices arrive as data."""
    T = seq
    mmdt = F32R if xdt == F32 else xdt   # attention matmul dtype
    pjdt = BF16                          # projection matmul dtype
    KC = D // 128               # 8 contraction chunks for projections
    NQ = seq // 512             # 4 query blocks == 4 token quarters
    NKT = seq // 128            # 16 k tiles of 128
    HKQ = 4                     # k tiles per quarter
    nc = bacc.Bacc()

    xq = nc.declare_dram_parameter("xq", [D, T], pjdt, isOutput=False)
    xk = nc.declare_dram_parameter("xk", [D, T], pjdt, isOutput=False)
    xv = nc.declare_dram_parameter("xv", [D, T], pjdt, isOutput=False)
    wq = nc.declare_dram_parameter("wq", [D, DH], pjdt, isOutput=False)
    wk = nc.declare_dram_parameter("wk", [D, DH], pjdt, isOutput=False)
    wv = nc.declare_dram_parameter("wv", [D, DH], pjdt, isOutput=False)
    wqb = nc.declare_dram_parameter("wqb", [128, G], F32, isOutput=False)
    wkb = nc.declare_dram_parameter("wkb", [128, G], F32, isOutput=False)
    wvb = nc.declare_dram_parameter("wvb", [128, G], F32, isOutput=False)
    wo = nc.declare_dram_parameter("wo", [DH, D], pjdt, isOutput=False)
    tri = nc.declare_dram_parameter("tri", [128, 128], mmdt, isOutput=False)
    idn = nc.declare_dram_parameter("idn", [128, 128], pjdt, isOutput=False)
    onesm = nc.declare_dram_parameter("onesm", [128, 512], mmdt, isOutput=False)
    onesr = nc.declare_dram_parameter("onesr", [1, DK], F32R, isOutput=False)
    madd = None
    if mode == "general":
        madd = nc.declare_dram_parameter("madd", [seq, seq], F32, isOutput=False)
    out = nc.declare_dram_parameter("out", [T, D], BF16, isOutput=True)
    # last block's group-1 partial (host adds it into rows [T-512, T))
    out2 = nc.declare_dram_parameter("out2", [512, D], BF16, isOutput=True)

    with tile.TileContext(nc) as tc, ExitStack() as ctx:
        persist = ctx.enter_context(tc.tile_pool(name="persist", bufs=1))
        wpool = ctx.enter_context(tc.tile_pool(name="wpool", bufs=1))
        xs0 = ctx.enter_context(tc.tile_pool(name="xs0", bufs=8))
        xsH = ctx.enter_context(tc.tile_pool(name="xsH", bufs=4))
        xsB = ctx.enter_context(tc.tile_pool(name="xsB", bufs=6))
        vts = ctx.enter_context(tc.tile_pool(name="vts", bufs=3))
        ptp = ctx.enter_context(tc.tile_pool(name="ptp", bufs=6))
        otn_p = ctx.enter_context(tc.tile_pool(name="otn", bufs=6))
        rc_p = ctx.enter_context(tc.tile_pool(name="rc", bufs=6))
        out_p = ctx.enter_context(tc.tile_pool(name="outp", bufs=6))
        mk_p = None
        if mode == "general":
            mk_p = ctx.enter_context(tc.tile_pool(name="mk", bufs=4))
        # PSUM: st2 2 bufs x 2 banks + otps 2 x 1 + po 2 x 1 = 8 banks
        st2 = ctx.enter_context(
            tc.tile_pool(name="st2", bufs=2, space=bass.MemorySpace.PSUM))
        otps = ctx.enter_context(
            tc.tile_pool(name="otps", bufs=2, space=bass.MemorySpace.PSUM))
        po = ctx.enter_context(
            tc.tile_pool(name="po", bufs=2, space=bass.MemorySpace.PSUM))

        # ---------------- persistent tiles ----------------
        # per-(quarter, group) tiles: attention consumers wait only on the
        # group slice they actually read
        qt_c = [[persist.tile([128, 512], mmdt, name=f"qt{i}g{g}")
                 for g in range(G)] for i in range(NQ)]
        kt_c = [[persist.tile([128, 512], mmdt, name=f"kt{i}g{g}")
                 for g in range(G)] for i in range(NQ)]
        # V augmented: [128 tokens, g*HKQ + ktile, head-in-group, 65]
        vaug_c = [persist.tile([128, G * HKQ, HPC, DK + 1], mmdt,
                               name=f"vaug{i}") for i in range(NQ)]
        wo_sb = persist.tile([128, G, D], pjdt)
        tri_sb = persist.tile([128, 128], mmdt)
        ident = persist.tile([128, 128], pjdt)
        ones_sb = persist.tile([128, 512], mmdt)
        onesr_sb = persist.tile([1, DK], F32R)

        # ---------------- weight / constant DMAs ----------------
        # wv rides the sync queue AHEAD of the x stream (V projects first
        # in the pre-attention quarter); everything else rides the gpsimd
        # queue, ordered by first use.
        wsrc_d = {"q": (xq, wq, qt_c), "k": (xk, wk, kt_c),
                  "v": (xv, wv, None)}
        w_sb = {}
        wb_sb = {}
        for name in ("q", "k", "v"):
            w_sb[name] = wpool.tile([128, KC, DH], pjdt, tag=f"w{name}",
                                    name=f"w{name}")
            wb_sb[name] = wpool.tile([128, G], F32, tag=f"wb{name}",
                                     name=f"wb{name}")
        nc.sync.dma_start(
            out=w_sb["v"],
            in_=wv[:, :].rearrange("(c p) n -> p c n", p=128))
        for name, bsrc in (("v", wvb), ("k", wkb), ("q", wqb)):
            nc.gpsimd.dma_start(out=wb_sb[name], in_=bsrc[:, :])
        nc.gpsimd.dma_start(out=ones_sb, in_=onesm[:, :])
        nc.gpsimd.dma_start(out=ident, in_=idn[:, :])
        nc.gpsimd.dma_start(
            out=w_sb["k"],
            in_=wk[:, :].rearrange("(c p) n -> p c n", p=128))
        nc.gpsimd.dma_start(out=tri_sb, in_=tri[:, :])
        nc.gpsimd.dma_start(
            out=w_sb["q"],
            in_=wq[:, :].rearrange("(c p) n -> p c n", p=128))
        nc.gpsimd.dma_start(out=onesr_sb, in_=onesr[:, :])
        nc.gpsimd.dma_start(
            out=wo_sb, in_=wo[:, :].rearrange("(g p) n -> p g n", p=128))

        # ---------------- x input streaming ----------------
        xt_access = {}       # (name, Q) -> fn(c) -> AP of chunk c

        def emit_x_dmas(Q, parts):
            """Issue quarter Q's x DMAs on the sync queue.  Quarter 0 is
            split finer (per-chunk / half) so the first projections can
            start while data streams in; later quarters use one large
            rearranged DMA per tensor to save issue time."""
            for name in parts:
                xsrc = wsrc_d[name][0]
                if Q == 0 and name == "v":
                    ts = []
                    for c in range(KC):
                        t = xs0.tile([128, 512], pjdt, tag="x0")
                        nc.sync.dma_start(
                            out=t, in_=xsrc[c * 128:(c + 1) * 128, 0:512])
                        ts.append(t)
                    xt_access[(name, Q)] = lambda c, ts=ts: ts[c]
                elif Q == 0:
                    # xq rides the gpsimd queue (after the weights) so it
                    # streams in parallel with sync's xv/xk
                    deng = nc.gpsimd if name == "q" else nc.sync
                    hs = []
                    for hh in range(2):
                        t = xsH.tile([128, KC // 2, 512], pjdt, tag="xh")
                        deng.dma_start(
                            out=t,
                            in_=xsrc[hh * 512:(hh + 1) * 512, 0:512]
                            .rearrange("(c p) t -> p c t", p=128))
                        hs.append(t)
                    xt_access[(name, Q)] = (
                        lambda c, hs=hs: hs[c // 4][:, c % 4, :])
                else:
                    t = xsB.tile([128, KC, 512], pjdt, tag="xb")
                    nc.sync.dma_start(
                        out=t,
                        in_=xsrc[:, Q * 512:(Q + 1) * 512]
                        .rearrange("(c p) t -> p c t", p=128))
                    xt_access[(name, Q)] = lambda c, t=t: t[:, c, :]

        def proj_gen(Q, parts, on_act):
            """Generator emitting quarter Q's projections one unit at a
            time.  Yields "safe" where no PSUM accumulation is open."""
            for name in parts:
                wt, bt = w_sb[name], wb_sb[name]
                xap = xt_access[(name, Q)]
                vtile = None
                if name == "v":
                    vtile = vts.tile([128, G, 512], pjdt, tag="vt")
                for g in range(G):
                    ps = po.tile([128, 512], F32, tag="po")
                    for c in range(KC):
                        nc.tensor.matmul(
                            ps, wt[:, c, g * 128:(g + 1) * 128], xap(c),
                            start=(c == 0), stop=(c == KC - 1))
                        yield None
                    if name == "v":
                        tgt = vtile[:, g, :]
                    else:
                        tgt = wsrc_d[name][2][Q][g][:, :]
                    if on_act:
                        nc.scalar.activation(
                            tgt, ps, mybir.ActivationFunctionType.Identity,
                            bias=bt[:, g:g + 1])
                    else:
                        # GpSimd cannot read PSUM; DVE drains the filler
                        nc.vector.tensor_scalar_add(tgt, ps, bt[:, g:g + 1])
                    yield "safe"
                if name == "v":
                    nc.vector.tensor_copy(
                        vaug_c[Q][:, :, :, DK:DK + 1],
                        ones_sb[:, 0:G * HKQ * HPC])
                    yield "safe"
                    for g in range(G):
                        for i in range(HKQ):
                            trp = po.tile([128, HPC, DK], pjdt, tag="po")
                            nc.tensor.transpose(
                                trp, vtile[:, g, i * 128:(i + 1) * 128],
                                ident)
                            yield None
                            nc.vector.tensor_copy(
                                vaug_c[Q][:, g * HKQ + i, :, 0:DK], trp)
                            yield "safe"

        # ---------------- attention ----------------
        def emit_scores(qj, g, ki):
            """Score matmuls for one 128-key tile; returns (st, off)."""
            off = 4 * (ki - 4 * qj) * 32 if (mode == "causal" and ki >= 4 * qj) else 0
            kh, kbase = ki // HKQ, (ki % HKQ) * 128
            st = st2.tile([128, 1024], F32, tag="st2")
            for h in range(HPC):
                nc.tensor.matmul(
                    st[:, h * 512 + off:(h + 1) * 512],
                    kt_c[kh][g][h * DK:(h + 1) * DK, kbase:kbase + 128],
                    qt_c[qj][g][h * DK:(h + 1) * DK, off:512],
                    start=True, stop=True,
                    tile_position=(h * DK, 0))
            if mode == "general":
                mt = mk_p.tile([128, 512], F32, tag="mk")
                nc.sync.dma_start(
                    out=mt,
                    in_=madd[ki * 128:(ki + 1) * 128,
                             qj * 512:(qj + 1) * 512])
                for h in range(HPC):
                    nc.vector.tensor_add(
                        st[:, h * 512:(h + 1) * 512],
                        st[:, h * 512:(h + 1) * 512], mt)
            return st, off

        def attn_ki_stream(qj, g, ots, filler, pending):
            """Pipelined score/exp/PV stream for one (qj, group).
            Emits S(ki+1) before PV(ki) so the PE never waits on exp;
            pumps filler between steps; emits the deferred `pending` job
            at ki==1 (at a filler-safe point)."""
            n_k = 4 * qj + 4 if mode == "causal" else NKT
            ots[g] = [otps.tile([DK + 1, 512], F32, tag="ot",
                                name=f"ot{_h}") for _h in range(HPC)]
            ot = ots[g]
            pend_s = emit_scores(qj, g, 0)
            for ki in range(n_k):
                st, off = pend_s
                pt = ptp.tile([128, 1024], mmdt, tag="pt")
                if off == 0:
                    nc.scalar.activation(
                        pt, st, mybir.ActivationFunctionType.Exp, scale=SCALE)
                else:
                    for h in range(HPC):
                        lo = h * 512
                        nc.scalar.activation(
                            pt[:, lo + off:lo + 512], st[:, lo + off:lo + 512],
                            mybir.ActivationFunctionType.Exp, scale=SCALE)
                # lookahead: next scores enter the PE queue before PV(ki)
                if ki + 1 < n_k:
                    if ki + 1 == 4 * qj:
                        # about to read this block's own quarter
                        filler.guard(qj)
                    pend_s = emit_scores(qj, g, ki + 1)
                if mode == "causal" and ki >= 4 * qj:
                    # pt/tri are SBUF-only -> GpSimd masks them, keeping
                    # DVE free for the psum drains
                    for h in range(HPC):
                        lo = h * 512 + off
                        nc.gpsimd.tensor_mul(
                            pt[:, lo:lo + 128], pt[:, lo:lo + 128], tri_sb)
                # filler goes on the PE queue BEFORE PV(ki): if exp(ki) is
                # still running, the PE does filler work instead of stalling
                filler.pump(3)
                kh, vs = ki // HKQ, ki % HKQ
                for h in range(HPC):
                    nc.tensor.matmul(
                        ot[h][:, off:512] if off else ot[h],
                        vaug_c[kh][:, g * HKQ + vs, h, :],
                        pt[:, h * 512 + off:(h + 1) * 512],
                        start=(ki == 0), stop=(ki == n_k - 1))
                if ki == 1 and pending[0] is not None:
                    filler.pump_to_safe()
                    pending[0]()
                    pending[0] = None

        def attn_norm(qj, g, ots, filler):
            """Normalize one finished group; frees its PV psum tiles.
            Filler is pumped between the reciprocal chain and the rbc
            matmuls so the PE has work during the DVE latency."""
            ot = ots[g]
            rrows = []
            for h in range(HPC):
                srow = rc_p.tile([1, 512], F32R, tag="srow",
                                 name=f"srow{h}")
                nc.vector.tensor_copy(srow, ot[h][DK:DK + 1, :])
                rrow_r = rc_p.tile([1, 512], F32R, tag="rrow_r",
                                   name=f"rrow_r{h}")
                _rc = RECIP_APPROX_FAST_CONSTS
                nc.vector._custom_dve(
                    RECIPROCAL_APPROX_FAST, out=rrow_r, in0=srow,
                    s0=_rc["s0"], s1=_rc["s1"], imm2=_rc["imm2"])
                rrows.append(rrow_r)
            filler.pump(4)
            filler.pump_to_safe()
            rbcs = []
            for h in range(HPC):
                rbc_ps = po.tile([DK, 512], F32, tag="po")
                nc.tensor.matmul(rbc_ps, onesr_sb, rrows[h],
                                 start=True, stop=True)
                rbc = rc_p.tile([DK, 512], F32, tag="rbc",
                                name=f"rbc{h}")
                nc.vector.tensor_copy(rbc, rbc_ps)
                rbcs.append(rbc)
            otn = otn_p.tile([128, 512], pjdt, tag="otn", name=f"otn{g}")
            for h in range(HPC):
                nc.vector.tensor_mul(
                    otn[h * DK:(h + 1) * DK, :], ot[h][0:DK, :], rbcs[h])
            return otn

        def attn_outproj(qj, otn_g):
            """Combined 256-feature output projection (blocks 0..NQ-2)."""
            qbase = qj * 512
            for qb in range(4):
                pts = [po.tile([128, 512], F32, tag="po", name=f"pts{_n}")
                       for _n in range(2)]
                for n in range(2):
                    for g in range(G):
                        nc.tensor.matmul(
                            pts[n],
                            otn_g[g][:, qb * 128:(qb + 1) * 128],
                            wo_sb[:, g, n * 512:(n + 1) * 512],
                            start=(g == 0), stop=(g == G - 1))
                ob = out_p.tile([128, 1024], BF16, tag="ob")
                # both copies on DVE: this runs inside an exp-paced stream,
                # so ACT must stay exp-only
                nc.vector.tensor_copy(ob[:, 0:512], pts[0])
                nc.vector.tensor_copy(ob[:, 512:1024], pts[1])
                nc.gpsimd.dma_start(
                    out=out[qbase + qb * 128:qbase + (qb + 1) * 128, :],
                    in_=ob)

        def outproj_partial(qj, otn, g, part):
            """Single-group output projection for the LAST block.  part 0
            stores g0's partial into `out` rows (g1's lands in `out2`;
            the host adds them), so g0's drain overlaps g1's stream."""
            qbase = qj * 512
            for qb in range(4):
                pts = [po.tile([128, 512], F32, tag="po", name=f"pp{_n}")
                       for _n in range(2)]
                for n in range(2):
                    nc.tensor.matmul(
                        pts[n],
                        otn[:, qb * 128:(qb + 1) * 128],
                        wo_sb[:, g, n * 512:(n + 1) * 512],
                        start=True, stop=True)
                ob = out_p.tile([128, 1024], BF16, tag="ob")
                if part == 0:
                    # during g1's stream: DVE-only copies, ACT stays exp-only
                    nc.vector.tensor_copy(ob[:, 0:512], pts[0])
                    nc.vector.tensor_copy(ob[:, 512:1024], pts[1])
                    nc.gpsimd.dma_start(
                        out=out[qbase + qb * 128:qbase + (qb + 1) * 128, :],
                        in_=ob)
                else:
                    # end of kernel: ACT is idle, split copies across engines
                    nc.vector.tensor_copy(ob[:, 0:512], pts[0])
                    nc.scalar.copy(ob[:, 512:1024], pts[1])
                    eng = nc.sync if qb % 2 == 0 else nc.gpsimd
                    eng.dma_start(
                        out=out2[qb * 128:(qb + 1) * 128, :], in_=ob)

        def attn_block(qj, filler, pending):
            last = qj == NQ - 1
            ots = {}
            otn_g = []
            attn_ki_stream(qj, 0, ots, filler, pending)
            filler.pump_to_safe()
            otn_g.append(attn_norm(qj, 0, ots, filler))
            if last:
                otn0 = otn_g[0]
                pending[0] = lambda: outproj_partial(qj, otn0, 0, 0)
            attn_ki_stream(qj, 1, ots, filler, pending)
            filler.pump_to_safe()
            otn_g.append(attn_norm(qj, 1, ots, filler))
            if last:
                if pending[0] is not None:     # n_k tiny safeguard
                    pending[0]()
                    pending[0] = None
                outproj_partial(qj, otn_g[1], 1, 1)
            else:
                pending[0] = lambda: attn_outproj(qj, otn_g)

        # ---------------- schedule ----------------
        emit_x_dmas(0, ("v", "k", "q"))
        boot = _Filler()
        boot.now = [(0, proj_gen(0, ("v", "k", "q"), on_act=True))]
        boot.flush_now()
        filler = _Filler()
        pending = [None]
        for qj in range(NQ):
            if qj + 1 < NQ:
                emit_x_dmas(qj + 1, ("q", "k", "v"))
                filler.now.append(
                    (qj + 1, proj_gen(qj + 1, ("q",), on_act=False)))
                filler.spill.append(
                    (qj + 1, proj_gen(qj + 1, ("k", "v"), on_act=False)))
            attn_block(qj, filler, pending)
            filler.rotate()
        if pending[0] is not None:
            pending[0]()
    nc.compile()
    return nc


def _np_dt(xdt):
    return np.float32 if xdt == F32 else ml_dtypes.bfloat16


def make_core_inputs(query, key, value, mask, Wq, bq, Wk, bk, Wv, bv, Wo, bo,
                     seq=S, mode="causal", xdt=BF16):
    """Host-side sharding: returns list of per-core input dicts."""
    ndt = _np_dt(xdt)
    pdt = ml_dtypes.bfloat16
    xq_b = [np.ascontiguousarray(query[b].reshape(seq, D).T.astype(pdt))
            for b in range(B)]
    xk_b = [np.ascontiguousarray(key[b].reshape(seq, D).T.astype(pdt))
            for b in range(B)]
    xv_b = [np.ascontiguousarray(value[b].reshape(seq, D).T.astype(pdt))
            for b in range(B)]
    tri = np.ascontiguousarray(np.triu(np.ones((128, 128), np.float32))).astype(ndt)
    in_maps = []
    for c in range(NCORES):
        b = c // CPB
        hq = c % CPB
        hsl = slice(DH * hq, DH * (hq + 1))
        m = {
            "xq": xq_b[b], "xk": xk_b[b], "xv": xv_b[b],
            "wq": np.ascontiguousarray(Wq[hsl, :].T.astype(pdt)),
            "wk": np.ascontiguousarray(Wk[hsl, :].T.astype(pdt)),
            "wv": np.ascontiguousarray(Wv[hsl, :].T.astype(pdt)),
            "wqb": np.ascontiguousarray(
                bq[hsl].astype(np.float32).reshape(G, 128).T),
            "wkb": np.ascontiguousarray(
                bk[hsl].astype(np.float32).reshape(G, 128).T),
            "wvb": np.ascontiguousarray(
                bv[hsl].astype(np.float32).reshape(G, 128).T),
            "wo": np.ascontiguousarray(Wo[:, hsl].T.astype(pdt)),
            "tri": tri,
            "idn": np.ascontiguousarray(np.eye(128, dtype=np.float32)).astype(pdt),
            "onesm": np.ones((128, 512), ndt),
            "onesr": np.ones((1, DK), np.float32),
        }
        if mode == "general":
            m["madd"] = np.ascontiguousarray(
                np.where(np.asarray(mask)[0, 0].T == 0, np.float32(-1e30),
                         np.float32(0.0)).astype(np.float32))
        in_maps.append(m)
    return in_maps


def detect_mode(mask, seq=S):
    m = np.asarray(mask)[0, 0]
    if (m == np.tril(np.ones((seq, seq), m.dtype))).all():
        return "causal"
    if (m == 1).all():
        return "ones"
    return "general"


_NC_CACHE = {}


def kernel(query, key, value, mask, Wq, bq, Wk, bk, Wv, bv, Wo, bo,
           xdt=BF16, trace=False):
    from concourse.bass_utils import run_bass_kernel_spmd

    query = np.asarray(query)
    mode = detect_mode(mask)
    key_ = (S, mode, xdt)
    if key_ not in _NC_CACHE:
        _NC_CACHE[key_] = build_kernel(seq=S, mode=mode, xdt=xdt)
    nc = _NC_CACHE[key_]
    in_maps = make_core_inputs(
        np.asarray(query), np.asarray(key), np.asarray(value), mask,
        np.asarray(Wq), np.asarray(bq), np.asarray(Wk), np.asarray(bk),
        np.asarray(Wv), np.asarray(bv), np.asarray(Wo), np.asarray(bo),
        seq=S, mode=mode, xdt=xdt)
    res = run_bass_kernel_spmd(nc, in_maps, core_ids=list(range(NCORES)),
                               trace=trace)
    acc = np.zeros((B, S, D), np.float64)
    for c, r in enumerate(res.results):
        acc[c // CPB] += r["out"].astype(np.float64)
        acc[c // CPB, S - 512:S, :] += r["out2"].astype(np.float64)
    acc += np.asarray(bo).astype(np.float64)[None, None, :]
    out = acc.astype(np.float32)
    if trace:
        kernel.last_results = res
    return out


# revision 24
# speedup vs baseline: 1.0767x; 1.0327x over previous
"""
Multi-head attention (B=2, S=2048, D=1024, H=16, causal mask) on 8 Trainium2
NeuronCores via Bass/Tile.

Sharding: batch x heads (data + tensor parallel) -- core c owns batch c//4
and the 4 heads [4*(c%4), 4*(c%4)+4) of that batch.  Each core reads only
its batch's activations, computes Q/K/V projections for its 256 features,
runs causal attention for its 4 heads (as two 128-feature head-pair groups),
and produces a partial output projection [2048, 1024].  The host sums the
partials per batch (plus the last-block g1 partial `out2`) and adds the
output bias.

Schedule (v3): software-pipelined around the pacing engines:
  * PE inner loop uses a one-step score LOOKAHEAD: S(ki+1) is emitted
    before PV(ki), so the in-order PE queue never head-of-line blocks on
    the ACT exp of the current tile.
  * Projections run at 512-token QUARTER granularity, woven into the
    attention blocks as PE filler.  The q-part of quarter Q+1 must finish
    inside block Q; the k/v parts may SPILL into block Q+1 (flushed by a
    guard before the stream first reads that quarter).  Filler yields
    "safe" markers at points with no open PSUM accumulation; other users
    of the same PSUM pool only emit at safe points (deadlock avoidance
    for the in-order engine queues).
  * x tiles arrive via few, large rearranged DMAs (DMA-issue instructions
    cost ~0.6us each on the issuing engine, so issue count is the real
    input-stream pacer).  Quarter 0 streams at finer granularity so the
    first projections can start as data trickles in.
  * The deferred output projection of block qj is emitted at ki==1 of
    block qj+1's first stream; the last block's outproj is split per
    group (g0's partial store lands during g1's stream; g1's partial
    goes to `out2`), halving the end-of-kernel drain.
  * ACT is exp-only during attention (projection psum drains ride DVE,
    causal masking rides GpSimd, which cannot touch PSUM but can mask
    SBUF pt tiles).

On-chip layouts (per core):
  Q_T, K_T : [128 feats (2 heads x 64), group g, 512 tokens] per quarter
  V        : vaug [128 tokens, g*4+ktile, head, 65]; col 64 == 1.0
             so the P@V matmul also produces the softmax row sums
  S_T      : scores^T tiles [128 keys, q]
  softmax  : exp on ACT (scale=1/8 folded; no max-subtraction needed),
             sums via the ones column of V, fast approx-reciprocal on
             DVE, partition-broadcast via a K=1 ones matmul on PE,
             normalize fused into the psum->sbuf move.
  out-proj : both heads of a group packed into one [128, 512] otn tile;
             the two groups accumulate into the same psum tile (except
             the split last block).
"""

import os
import sys

for _p in ("/opt/trn_rl_repo", "/root/.axon_site/_ro/trn_rl_repo"):
    if os.path.isdir(_p) and _p not in sys.path:
        sys.path.insert(0, _p)

import numpy as np
import ml_dtypes
from contextlib import ExitStack

import concourse.bass as bass
import concourse.tile as tile
from concourse import bacc
from concourse import mybir
from concourse.dve_ops import (
    RECIP_APPROX_FAST_CONSTS,
    RECIPROCAL_APPROX_FAST,
)

B, S, D, H = 2, 2048, 1024, 16
DK = D // H            # 64
NCORES = 8
BGROUPS = 2            # batch groups
CPB = NCORES // BGROUPS    # cores per batch = 4
HPC_TOT = H // CPB     # 4 heads per core
G = 2                  # head-pair groups per core
HPC = HPC_TOT // G     # 2 heads per group
DH = HPC_TOT * DK      # 256 features per core
SCALE = 1.0 / np.sqrt(DK)  # 0.125

F32 = mybir.dt.float32
F32R = mybir.dt.float32r
BF16 = mybir.dt.bfloat16


class _Filler:
    """Priority queue of projection generators used as PE filler.

    `now` entries must finish within the current attention block;
    `spill` entries may run ahead opportunistically and roll over.
    Each entry is (quarter_tag, generator).  Generators yield "safe"
    when they hold no open PSUM accumulation."""

    def __init__(self):
        self.now = []
        self.spill = []
        self.marker = "safe"

    def _pump_one(self):
        while True:
            if self.now:
                src, gen = self.now, self.now[0][1]
            elif self.spill:
                src, gen = self.spill, self.spill[0][1]
            else:
                return False
            try:
                self.marker = next(gen)
                return True
            except StopIteration:
                self.marker = "safe"
                src.pop(0)

    def pump(self, n):
        for _ in range(n):
            if not self._pump_one():
                return

    def pump_to_safe(self):
        while self.marker != "safe":
            if not self._pump_one():
                return

    def flush_now(self):
        while self.now:
            if not self._pump_one():
                break
        self.pump_to_safe()

    def guard(self, qj):
        """Finish every now-generator tagged <= qj (its outputs are about
        to be read by the stream)."""
        while self.now and self.now[0][0] <= qj:
            gen = self.now[0][1]
            try:
                while True:
                    self.marker = next(gen)
            except StopIteration:
                self.marker = "safe"
                self.now.pop(0)

    def rotate(self):
        self.flush_now()
        self.now = self.spill
        self.spill = []


def build_kernel(seq=S, mode="causal", xdt=BF16, dbg=False):
    """Build the per-core Bass program.  Identical program on all cores;
    per-core batch/head slices arrive as data."""
    T = seq
    mmdt = F32R if xdt == F32 else xdt   # attention matmul dtype
    pjdt = BF16                          # projection matmul dtype
    KC = D // 128               # 8 contraction chunks for projections
    NQ = seq // 512             # 4 query blocks == 4 token quarters
    NKT = seq // 128            # 16 k tiles of 128
    HKQ = 4                     # k tiles per quarter
    nc = bacc.Bacc()

    xq = nc.declare_dram_parameter("xq", [D, T], pjdt, isOutput=False)
    xk = nc.declare_dram_parameter("xk", [D, T], pjdt, isOutput=False)
    xv = nc.declare_dram_parameter("xv", [D, T], pjdt, isOutput=False)
    wq = nc.declare_dram_parameter("wq", [D, DH], pjdt, isOutput=False)
    wk = nc.declare_dram_parameter("wk", [D, DH], pjdt, isOutput=False)
    wv = nc.declare_dram_parameter("wv", [D, DH], pjdt, isOutput=False)
    wqb = nc.declare_dram_parameter("wqb", [128, G], F32, isOutput=False)
    wkb = nc.declare_dram_parameter("wkb", [128, G], F32, isOutput=False)
    wvb = nc.declare_dram_parameter("wvb", [128, G], F32, isOutput=False)
    wo = nc.declare_dram_parameter("wo", [DH, D], pjdt, isOutput=False)
    tri = nc.declare_dram_parameter("tri", [128, 128], mmdt, isOutput=False)
    idn = nc.declare_dram_parameter("idn", [128, 128], pjdt, isOutput=False)
    onesm = nc.declare_dram_parameter("onesm", [128, 512], mmdt, isOutput=False)
    onesr = nc.declare_dram_parameter("onesr", [1, DK], F32R, isOutput=False)
    madd = None
    if mode == "general":
        madd = nc.declare_dram_parameter("madd", [seq, seq], F32, isOutput=False)
    out = nc.declare_dram_parameter("out", [T, D], BF16, isOutput=True)
    # last block's group-1 partial (host adds it into rows [T-512, T))
    out2 = nc.declare_dram_parameter("out2", [512, D], BF16, isOutput=True)

    with tile.TileContext(nc) as tc, ExitStack() as ctx:
        persist = ctx.enter_context(tc.tile_pool(name="persist", bufs=1))
        wpool = ctx.enter_context(tc.tile_pool(name="wpool", bufs=1))
        xs0 = ctx.enter_context(tc.tile_pool(name="xs0", bufs=8))
        xsH = ctx.enter_context(tc.tile_pool(name="xsH", bufs=4))
        xsB = ctx.enter_context(tc.tile_pool(name="xsB", bufs=12))
        vts = ctx.enter_context(tc.tile_pool(name="vts", bufs=3))
        ptp = ctx.enter_context(tc.tile_pool(name="ptp", bufs=6))
        otn_p = ctx.enter_context(tc.tile_pool(name="otn", bufs=6))
        rc_p = ctx.enter_context(tc.tile_pool(name="rc", bufs=6))
        out_p = ctx.enter_context(tc.tile_pool(name="outp", bufs=6))
        mk_p = None
        if mode == "general":
            mk_p = ctx.enter_context(tc.tile_pool(name="mk", bufs=4))
        # PSUM: st2 2 bufs x 2 banks + otps 2 x 1 + po 2 x 1 = 8 banks
        st2 = ctx.enter_context(
            tc.tile_pool(name="st2", bufs=2, space=bass.MemorySpace.PSUM))
        otps = ctx.enter_context(
            tc.tile_pool(name="otps", bufs=2, space=bass.MemorySpace.PSUM))
        po = ctx.enter_context(
            tc.tile_pool(name="po", bufs=2, space=bass.MemorySpace.PSUM))

        # ---------------- persistent tiles ----------------
        # per-(quarter, group) tiles: attention consumers wait only on the
        # group slice they actually read
        qt_c = [[persist.tile([128, 512], mmdt, name=f"qt{i}g{g}")
                 for g in range(G)] for i in range(NQ)]
        kt_c = [[persist.tile([128, 512], mmdt, name=f"kt{i}g{g}")
                 for g in range(G)] for i in range(NQ)]
        # V augmented: [128 tokens, g*HKQ + ktile, head-in-group, 65]
        vaug_c = [persist.tile([128, G * HKQ, HPC, DK + 1], mmdt,
                               name=f"vaug{i}") for i in range(NQ)]
        wo_sb = persist.tile([128, G, D], pjdt)
        tri_sb = persist.tile([128, 128], mmdt)
        ident = persist.tile([128, 128], pjdt)
        ones_sb = persist.tile([128, 512], mmdt)
        onesr_sb = persist.tile([1, DK], F32R)

        # ---------------- weight / constant DMAs ----------------
        # wv rides the sync queue AHEAD of the x stream (V projects first
        # in the pre-attention quarter); everything else rides the gpsimd
        # queue, ordered by first use.
        wsrc_d = {"q": (xq, wq, qt_c), "k": (xk, wk, kt_c),
                  "v": (xv, wv, None)}
        w_sb = {}
        wb_sb = {}
        for name in ("q", "k", "v"):
            w_sb[name] = wpool.tile([128, KC, DH], pjdt, tag=f"w{name}",
                                    name=f"w{name}")
            wb_sb[name] = wpool.tile([128, G], F32, tag=f"wb{name}",
                                     name=f"wb{name}")
        nc.sync.dma_start(
            out=w_sb["v"],
            in_=wv[:, :].rearrange("(c p) n -> p c n", p=128))
        for name, bsrc in (("v", wvb), ("k", wkb), ("q", wqb)):
            nc.gpsimd.dma_start(out=wb_sb[name], in_=bsrc[:, :])
        nc.gpsimd.dma_start(out=ones_sb, in_=onesm[:, :])
        nc.gpsimd.dma_start(out=ident, in_=idn[:, :])
        nc.gpsimd.dma_start(
            out=w_sb["k"],
            in_=wk[:, :].rearrange("(c p) n -> p c n", p=128))
        nc.gpsimd.dma_start(out=tri_sb, in_=tri[:, :])
        nc.gpsimd.dma_start(
            out=w_sb["q"],
            in_=wq[:, :].rearrange("(c p) n -> p c n", p=128))
        nc.gpsimd.dma_start(out=onesr_sb, in_=onesr[:, :])
        nc.gpsimd.dma_start(
            out=wo_sb, in_=wo[:, :].rearrange("(g p) n -> p g n", p=128))

        # ---------------- x input streaming ----------------
        xt_access = {}       # (name, Q) -> fn(c) -> AP of chunk c

        def emit_x_dmas(Q, parts):
            """Issue quarter Q's x DMAs on the sync queue.  Quarter 0 is
            split finer (per-chunk / half) so the first projections can
            start while data streams in; later quarters use one large
            rearranged DMA per tensor to save issue time."""
            for name in parts:
                xsrc = wsrc_d[name][0]
                if Q == 0 and name == "v":
                    ts = []
                    for c in range(KC):
                        t = xs0.tile([128, 512], pjdt, tag="x0")
                        nc.sync.dma_start(
                            out=t, in_=xsrc[c * 128:(c + 1) * 128, 0:512])
                        ts.append(t)
                    xt_access[(name, Q)] = lambda c, ts=ts: ts[c]
                elif Q == 0:
                    # xq rides the gpsimd queue (after the weights) so it
                    # streams in parallel with sync's xv/xk
                    deng = nc.gpsimd if name == "q" else nc.sync
                    hs = []
                    for hh in range(2):
                        t = xsH.tile([128, KC // 2, 512], pjdt, tag="xh")
                        deng.dma_start(
                            out=t,
                            in_=xsrc[hh * 512:(hh + 1) * 512, 0:512]
                            .rearrange("(c p) t -> p c t", p=128))
                        hs.append(t)
                    xt_access[(name, Q)] = (
                        lambda c, hs=hs: hs[c // 4][:, c % 4, :])
                else:
                    # two half-issues per tensor: few DMA-issue instructions
                    # (each costs ~0.6us of engine time) but fine enough
                    # arrival granularity that fillers never HOL-block the
                    # PE queue waiting for a full-quarter transfer
                    hs = []
                    for hh in range(2):
                        t = xsB.tile([128, KC // 2, 512], pjdt, tag="xb")
                        nc.sync.dma_start(
                            out=t,
                            in_=xsrc[hh * 512:(hh + 1) * 512,
                                     Q * 512:(Q + 1) * 512]
                            .rearrange("(c p) t -> p c t", p=128))
                        hs.append(t)
                    xt_access[(name, Q)] = (
                        lambda c, hs=hs: hs[c // 4][:, c % 4, :])

        def proj_gen(Q, parts, on_act):
            """Generator emitting quarter Q's projections one unit at a
            time.  Yields "safe" where no PSUM accumulation is open."""
            for name in parts:
                wt, bt = w_sb[name], wb_sb[name]
                xap = xt_access[(name, Q)]
                vtile = None
                if name == "v":
                    vtile = vts.tile([128, G, 512], pjdt, tag="vt")
                for g in range(G):
                    ps = po.tile([128, 512], F32, tag="po")
                    for c in range(KC):
                        nc.tensor.matmul(
                            ps, wt[:, c, g * 128:(g + 1) * 128], xap(c),
                            start=(c == 0), stop=(c == KC - 1))
                        yield None
                    if name == "v":
                        tgt = vtile[:, g, :]
                    else:
                        tgt = wsrc_d[name][2][Q][g][:, :]
                    if on_act:
                        nc.scalar.activation(
                            tgt, ps, mybir.ActivationFunctionType.Identity,
                            bias=bt[:, g:g + 1])
                    else:
                        # GpSimd cannot read PSUM; DVE drains the filler
                        nc.vector.tensor_scalar_add(tgt, ps, bt[:, g:g + 1])
                    yield "safe"
                if name == "v":
                    nc.vector.tensor_copy(
                        vaug_c[Q][:, :, :, DK:DK + 1],
                        ones_sb[:, 0:G * HKQ * HPC])
                    yield "safe"
                    for g in range(G):
                        for i in range(HKQ):
                            trp = po.tile([128, HPC, DK], pjdt, tag="po")
                            nc.tensor.transpose(
                                trp, vtile[:, g, i * 128:(i + 1) * 128],
                                ident)
                            yield None
                            nc.vector.tensor_copy(
                                vaug_c[Q][:, g * HKQ + i, :, 0:DK], trp)
                            yield "safe"

        # ---------------- attention ----------------
        def emit_scores(qj, g, ki):
            """Score matmuls for one 128-key tile; returns (st, off)."""
            off = 4 * (ki - 4 * qj) * 32 if (mode == "causal" and ki >= 4 * qj) else 0
            kh, kbase = ki // HKQ, (ki % HKQ) * 128
            st = st2.tile([128, 1024], F32, tag="st2")
            for h in range(HPC):
                nc.tensor.matmul(
                    st[:, h * 512 + off:(h + 1) * 512],
                    kt_c[kh][g][h * DK:(h + 1) * DK, kbase:kbase + 128],
                    qt_c[qj][g][h * DK:(h + 1) * DK, off:512],
                    start=True, stop=True,
                    tile_position=(h * DK, 0))
            if mode == "general":
                mt = mk_p.tile([128, 512], F32, tag="mk")
                nc.sync.dma_start(
                    out=mt,
                    in_=madd[ki * 128:(ki + 1) * 128,
                             qj * 512:(qj + 1) * 512])
                for h in range(HPC):
                    nc.vector.tensor_add(
                        st[:, h * 512:(h + 1) * 512],
                        st[:, h * 512:(h + 1) * 512], mt)
            return st, off

        def attn_ki_stream(qj, g, ots, filler, pending):
            """Pipelined score/exp/PV stream for one (qj, group).
            Emits S(ki+1) before PV(ki) so the PE never waits on exp;
            pumps filler between steps; emits the deferred `pending` job
            at ki==1 (at a filler-safe point)."""
            n_k = 4 * qj + 4 if mode == "causal" else NKT
            ots[g] = [otps.tile([DK + 1, 512], F32, tag="ot",
                                name=f"ot{_h}") for _h in range(HPC)]
            ot = ots[g]
            pend_s = emit_scores(qj, g, 0)
            for ki in range(n_k):
                st, off = pend_s
                pt = ptp.tile([128, 1024], mmdt, tag="pt")
                if off == 0:
                    nc.scalar.activation(
                        pt, st, mybir.ActivationFunctionType.Exp, scale=SCALE)
                else:
                    for h in range(HPC):
                        lo = h * 512
                        nc.scalar.activation(
                            pt[:, lo + off:lo + 512], st[:, lo + off:lo + 512],
                            mybir.ActivationFunctionType.Exp, scale=SCALE)
                # lookahead: next scores enter the PE queue before PV(ki)
                if ki + 1 < n_k:
                    if ki + 1 == 4 * qj:
                        # about to read this block's own quarter
                        filler.guard(qj)
                    pend_s = emit_scores(qj, g, ki + 1)
                if mode == "causal" and ki >= 4 * qj:
                    # pt/tri are SBUF-only -> GpSimd masks them, keeping
                    # DVE free for the psum drains
                    for h in range(HPC):
                        lo = h * 512 + off
                        nc.gpsimd.tensor_mul(
                            pt[:, lo:lo + 128], pt[:, lo:lo + 128], tri_sb)
                # filler goes on the PE queue BEFORE PV(ki): if exp(ki) is
                # still running, the PE does filler work instead of stalling
                filler.pump(3)
                kh, vs = ki // HKQ, ki % HKQ
                for h in range(HPC):
                    nc.tensor.matmul(
                        ot[h][:, off:512] if off else ot[h],
                        vaug_c[kh][:, g * HKQ + vs, h, :],
                        pt[:, h * 512 + off:(h + 1) * 512],
                        start=(ki == 0), stop=(ki == n_k - 1))
                # deferred outproj: one qb per odd step, so its DVE copies
                # never spike the queue that drains the filler psums
                if ki % 2 == 1 and pending[0]:
                    filler.pump_to_safe()
                    pending[0].pop(0)()

        def attn_norm(qj, g, ots, filler):
            """Normalize one finished group; frees its PV psum tiles.
            Filler is pumped between the reciprocal chain and the rbc
            matmuls so the PE has work during the DVE latency."""
            ot = ots[g]
            rrows = []
            for h in range(HPC):
                srow = rc_p.tile([1, 512], F32R, tag="srow",
                                 name=f"srow{h}")
                nc.vector.tensor_copy(srow, ot[h][DK:DK + 1, :])
                rrow_r = rc_p.tile([1, 512], F32R, tag="rrow_r",
                                   name=f"rrow_r{h}")
                _rc = RECIP_APPROX_FAST_CONSTS
                nc.vector._custom_dve(
                    RECIPROCAL_APPROX_FAST, out=rrow_r, in0=srow,
                    s0=_rc["s0"], s1=_rc["s1"], imm2=_rc["imm2"])
                rrows.append(rrow_r)
            filler.pump(4)
            filler.pump_to_safe()
            rbcs = []
            for h in range(HPC):
                rbc_ps = po.tile([DK, 512], F32, tag="po")
                nc.tensor.matmul(rbc_ps, onesr_sb, rrows[h],
                                 start=True, stop=True)
                rbc = rc_p.tile([DK, 512], F32, tag="rbc",
                                name=f"rbc{h}")
                nc.vector.tensor_copy(rbc, rbc_ps)
                rbcs.append(rbc)
            otn = otn_p.tile([128, 512], pjdt, tag="otn", name=f"otn{g}")
            for h in range(HPC):
                nc.vector.tensor_mul(
                    otn[h * DK:(h + 1) * DK, :], ot[h][0:DK, :], rbcs[h])
            return otn

        def outproj_qb(qj, otn_g, qb):
            """One 128-token slice of the combined output projection."""
            qbase = qj * 512
            pts = [po.tile([128, 512], F32, tag="po", name=f"pts{_n}")
                   for _n in range(2)]
            for n in range(2):
                for g in range(G):
                    nc.tensor.matmul(
                        pts[n],
                        otn_g[g][:, qb * 128:(qb + 1) * 128],
                        wo_sb[:, g, n * 512:(n + 1) * 512],
                        start=(g == 0), stop=(g == G - 1))
            ob = out_p.tile([128, 1024], BF16, tag="ob")
            # both copies on DVE: this runs inside an exp-paced stream,
            # so ACT must stay exp-only
            nc.vector.tensor_copy(ob[:, 0:512], pts[0])
            nc.vector.tensor_copy(ob[:, 512:1024], pts[1])
            nc.gpsimd.dma_start(
                out=out[qbase + qb * 128:qbase + (qb + 1) * 128, :],
                in_=ob)

        def partial_qb(qj, otn, g, part, qb):
            """One slice of the LAST block's single-group outproj.  part 0
            stores g0's partial into `out` rows (g1's lands in `out2`;
            the host adds them), so g0's drain overlaps g1's stream."""
            qbase = qj * 512
            pts = [po.tile([128, 512], F32, tag="po", name=f"pp{_n}")
                   for _n in range(2)]
            for n in range(2):
                nc.tensor.matmul(
                    pts[n],
                    otn[:, qb * 128:(qb + 1) * 128],
                    wo_sb[:, g, n * 512:(n + 1) * 512],
                    start=True, stop=True)
            ob = out_p.tile([128, 1024], BF16, tag="ob")
            if part == 0:
                # during g1's stream: DVE-only copies, ACT stays exp-only
                nc.vector.tensor_copy(ob[:, 0:512], pts[0])
                nc.vector.tensor_copy(ob[:, 512:1024], pts[1])
                nc.gpsimd.dma_start(
                    out=out[qbase + qb * 128:qbase + (qb + 1) * 128, :],
                    in_=ob)
            else:
                # end of kernel: ACT is idle, split copies across engines
                nc.vector.tensor_copy(ob[:, 0:512], pts[0])
                nc.scalar.copy(ob[:, 512:1024], pts[1])
                eng = nc.sync if qb % 2 == 0 else nc.gpsimd
                eng.dma_start(
                    out=out2[qb * 128:(qb + 1) * 128, :], in_=ob)

        def drain_pending(filler, pending):
            while pending[0]:
                filler.pump_to_safe()
                pending[0].pop(0)()

        def attn_block(qj, filler, pending):
            last = qj == NQ - 1
            ots = {}
            otn_g = []
            attn_ki_stream(qj, 0, ots, filler, pending)
            drain_pending(filler, pending)
            filler.pump_to_safe()
            otn_g.append(attn_norm(qj, 0, ots, filler))
            if last:
                otn0 = otn_g[0]
                pending[0] = [
                    (lambda qb=qb: partial_qb(qj, otn0, 0, 0, qb))
                    for qb in range(4)]
            attn_ki_stream(qj, 1, ots, filler, pending)
            drain_pending(filler, pending)
            filler.pump_to_safe()
            otn_g.append(attn_norm(qj, 1, ots, filler))
            if last:
                for qb in range(4):
                    partial_qb(qj, otn_g[1], 1, 1, qb)
            else:
                pending[0] = [
                    (lambda qb=qb: outproj_qb(qj, otn_g, qb))
                    for qb in range(4)]

        # ---------------- schedule ----------------
        # x DMAs for quarter Q are issued one block before its projection
        # generators run, so filler matmuls never wait on transfers.
        emit_x_dmas(0, ("v", "k", "q"))
        emit_x_dmas(1, ("q", "k", "v"))
        boot = _Filler()
        boot.now = [(0, proj_gen(0, ("v", "k", "q"), on_act=True))]
        boot.flush_now()
        filler = _Filler()
        pending = [[]]
        for qj in range(NQ):
            if qj + 2 < NQ:
                emit_x_dmas(qj + 2, ("q", "k", "v"))
            if qj + 1 < NQ:
                filler.now.append(
                    (qj + 1, proj_gen(qj + 1, ("q",), on_act=False)))
                filler.spill.append(
                    (qj + 1, proj_gen(qj + 1, ("k", "v"), on_act=False)))
            attn_block(qj, filler, pending)
            filler.rotate()
        drain_pending(filler, pending)
    nc.compile()
    return nc


def _np_dt(xdt):
    return np.float32 if xdt == F32 else ml_dtypes.bfloat16


def make_core_inputs(query, key, value, mask, Wq, bq, Wk, bk, Wv, bv, Wo, bo,
                     seq=S, mode="causal", xdt=BF16):
    """Host-side sharding: returns list of per-core input dicts."""
    ndt = _np_dt(xdt)
    pdt = ml_dtypes.bfloat16
    xq_b = [np.ascontiguousarray(query[b].reshape(seq, D).T.astype(pdt))
            for b in range(B)]
    xk_b = [np.ascontiguousarray(key[b].reshape(seq, D).T.astype(pdt))
            for b in range(B)]
    xv_b = [np.ascontiguousarray(value[b].reshape(seq, D).T.astype(pdt))
            for b in range(B)]
    tri = np.ascontiguousarray(np.triu(np.ones((128, 128), np.float32))).astype(ndt)
    in_maps = []
    for c in range(NCORES):
        b = c // CPB
        hq = c % CPB
        hsl = slice(DH * hq, DH * (hq + 1))
        m = {
            "xq": xq_b[b], "xk": xk_b[b], "xv": xv_b[b],
            "wq": np.ascontiguousarray(Wq[hsl, :].T.astype(pdt)),
            "wk": np.ascontiguousarray(Wk[hsl, :].T.astype(pdt)),
            "wv": np.ascontiguousarray(Wv[hsl, :].T.astype(pdt)),
            "wqb": np.ascontiguousarray(
                bq[hsl].astype(np.float32).reshape(G, 128).T),
            "wkb": np.ascontiguousarray(
                bk[hsl].astype(np.float32).reshape(G, 128).T),
            "wvb": np.ascontiguousarray(
                bv[hsl].astype(np.float32).reshape(G, 128).T),
            "wo": np.ascontiguousarray(Wo[:, hsl].T.astype(pdt)),
            "tri": tri,
            "idn": np.ascontiguousarray(np.eye(128, dtype=np.float32)).astype(pdt),
            "onesm": np.ones((128, 512), ndt),
            "onesr": np.ones((1, DK), np.float32),
        }
        if mode == "general":
            m["madd"] = np.ascontiguousarray(
                np.where(np.asarray(mask)[0, 0].T == 0, np.float32(-1e30),
                         np.float32(0.0)).astype(np.float32))
        in_maps.append(m)
    return in_maps


def detect_mode(mask, seq=S):
    m = np.asarray(mask)[0, 0]
    if (m == np.tril(np.ones((seq, seq), m.dtype))).all():
        return "causal"
    if (m == 1).all():
        return "ones"
    return "general"


_NC_CACHE = {}


def kernel(query, key, value, mask, Wq, bq, Wk, bk, Wv, bv, Wo, bo,
           xdt=BF16, trace=False):
    from concourse.bass_utils import run_bass_kernel_spmd

    query = np.asarray(query)
    mode = detect_mode(mask)
    key_ = (S, mode, xdt)
    if key_ not in _NC_CACHE:
        _NC_CACHE[key_] = build_kernel(seq=S, mode=mode, xdt=xdt)
    nc = _NC_CACHE[key_]
    in_maps = make_core_inputs(
        np.asarray(query), np.asarray(key), np.asarray(value), mask,
        np.asarray(Wq), np.asarray(bq), np.asarray(Wk), np.asarray(bk),
        np.asarray(Wv), np.asarray(bv), np.asarray(Wo), np.asarray(bo),
        seq=S, mode=mode, xdt=xdt)
    res = run_bass_kernel_spmd(nc, in_maps, core_ids=list(range(NCORES)),
                               trace=trace)
    acc = np.zeros((B, S, D), np.float64)
    for c, r in enumerate(res.results):
        acc[c // CPB] += r["out"].astype(np.float64)
        acc[c // CPB, S - 512:S, :] += r["out2"].astype(np.float64)
    acc += np.asarray(bo).astype(np.float64)[None, None, :]
    out = acc.astype(np.float32)
    if trace:
        kernel.last_results = res
    return out
